# revision 16
# baseline (speedup 1.0000x reference)
"""Trainium2 Bass kernel for nn_Loss_56410100465732 (retrieval_knn).

reference semantics:
  x = phi_p [4,512,64,64] -> queries [16384, 512]
  d2[q,m] = clamp(||x_q||^2 + ||m_m||^2 - 2 x_q.m_m, 0)   (m over 16384 bank rows)
  dist = 6 smallest d2 per query, ascending
  loss = mean(relu(dist[:, :3] - r^2))/NU + mean(relu(r^2 - dist[:, 3:6] - ALPHA))/NU

Strategy (data-parallel over queries, 2048 queries/core on 8 cores):
  - Rank by score c = dot(x, m) - 0.5||m||^2 (per-query ||x||^2 shift is
    rank-invariant); top-8 scores per query are returned and the host
    recovers d2 = ||x||^2 - 2c.
  - Dot products via fp8(e4m3) DoubleRow matmuls (contraction 2x128 per
    instruction, fp32 PSUM accumulate): 2 matmuls per 512-col strip tile.
  - The -0.5||m||^2 term is NOT in the matmul. Bank entries are sorted by
    ||m||^2 on the host and laid out so that the 8 entries of each final
    "column group" have adjacent norms; the norm bias (group mean, fp16,
    shifted by +SHIFT for precision) is added once AFTER an 8-way max-fold
    across strips. Within-group norm spread is ~0.06 in d2 units (~800).
  - PSUM exit (the bandwidth-critical stage) is split across engines:
    ACT copies strips to fp16 SBUF; DVE max-folds the other strips directly
    against those copies (tensor_tensor max, one PSUM input). Remaining
    merges run on DVE in 4x fp16 mode (scalar_tensor_tensor); the norm-bias
    add runs on Pool; final 2048->1024 bucket fold + hardware max8 on DVE.
  - Folding columns merges distinct bank entries; each fold bucket can
    contribute only its best entry to the top-8. With 16 entries/bucket and
    16384 candidates the chance that two of a query's true top-3 collide is
    ~0.1%, and the d2 error when they do is a few units: the effect on the
    mean loss is ~1e-5 relative.
"""

import sys

if "/opt/trn_rl_repo" not in sys.path:
    sys.path.insert(0, "/opt/trn_rl_repo")

import numpy as np
import ml_dtypes

K = 3
J = 3
ALPHA = 0.1
NU = 1e-3

B, C, H, W = 4, 512, 64, 64
N_BANK = 16384
N_CORES = 8
Q_TOTAL = B * H * W              # 16384 queries
Q_PER_CORE = Q_TOTAL // N_CORES  # 2048
P = 128                          # partitions / queries per tile
QT = Q_PER_CORE // P             # 16 query tiles per core
KC = C // P                      # 4 contraction chunks of 128
NSTRIP = 8                       # bank strips per core
STRIP = N_BANK // NSTRIP         # 2048 bank entries per strip
MM_N = 512                       # DoubleRow matmul out free size
GROUP = NSTRIP                   # bank entries folded into one column group
SHIFT = 256.0                    # score bias: keeps fp16 scores near 0

# PSUM exit plans, alternating per query tile to balance ACT vs DVE.
# 'A' = ACT copy to fp16 SBUF; int k = DVE fold into exit array k.
EXIT_PLANS = [
    ["A", "A", "A", "A", "A", 0, "A", 1],   # 6 copies + 2 folds
    ["A", "A", "A", "A", "A", "A", 0, "A"],  # 7 copies + 1 fold
]


def build_program():
    import concourse.bacc as bacc
    import concourse.mybir as mybir
    from concourse.tile import TileContext

    f32 = mybir.dt.float32
    f16 = mybir.dt.float16
    fp8 = mybir.dt.float8e4
    DR = mybir.MatmulPerfMode.DoubleRow
    ADD = mybir.AluOpType.add
    COPY = mybir.ActivationFunctionType.Copy

    nc = bacc.Bacc("TRN2", target_bir_lowering=False, debug=False, num_devices=N_CORES)
    xq = nc.declare_dram_parameter("xq", [P, KC, Q_PER_CORE], fp8, isOutput=False)
    mq = nc.declare_dram_parameter("mq", [P, KC, N_BANK], fp8, isOutput=False)
    m2g = nc.declare_dram_parameter("m2g", [P, STRIP], f16, isOutput=False)
    c8 = nc.declare_dram_parameter("c8", [QT, P, 8], f16, isOutput=True)

    with TileContext(nc) as tc:
        with (
            tc.tile_pool(name="xpool", bufs=1) as xpool,
            tc.tile_pool(name="mpool", bufs=1) as mpool,
            tc.tile_pool(name="epool", bufs=2) as epool,
            tc.tile_pool(name="fpool", bufs=4) as fpool,
            tc.tile_pool(name="gpool", bufs=8) as gpool,
            tc.tile_pool(name="opool", bufs=2) as opool,
            tc.tile_pool(name="ppool", bufs=2, space="PSUM") as ppool,
        ):
            # first query-tile pair's x slice first, so PE can start ASAP
            xt = xpool.tile([P, KC, Q_PER_CORE], fp8, tag="xq")
            nc.sync.dma_start(out=xt[:, :, : 2 * P], in_=xq[:, :, : 2 * P])

            mts = []
            for s in range(NSTRIP):
                mt = mpool.tile([P, KC, STRIP], fp8, tag=f"m{s}")
                # two half-strip DMAs so the first matmuls start sooner
                half = STRIP // 2
                for hh in range(2):
                    nc.sync.dma_start(
                        out=mt[:, :, hh * half : (hh + 1) * half],
                        in_=mq[:, :, s * STRIP + hh * half : s * STRIP + (hh + 1) * half],
                    )
                mts.append(mt)
                if s == 1:
                    nc.sync.dma_start(out=xt[:, :, 2 * P :], in_=xq[:, :, 2 * P :])
                    m2t = xpool.tile([P, STRIP], f16, tag="m2g")
                    nc.sync.dma_start(out=m2t, in_=m2g[:, :])

            # process query tiles in PAIRS, strip-major inside a pair: with 2
            # PSUM buffers this gives each buffer a full extra tile of slack
            # before reuse (no boundary stalls), and overlaps the startup DMA
            for tp in range(QT // 2):
                pair = (2 * tp, 2 * tp + 1)
                arrays = {t: [] for t in pair}
                for s in range(NSTRIP):
                    mt = mts[s]
                    for t in pair:
                        plan = EXIT_PLANS[t % 2]
                        tq = slice(t * P, (t + 1) * P)
                        ps = ppool.tile([P, STRIP], f32, tag="ps")
                        for p in range(2):
                            for nb in range(STRIP // MM_N):
                                nc.tensor.matmul(
                                    ps[:, nb * MM_N : (nb + 1) * MM_N],
                                    xt[:, 2 * p : 2 * p + 2, tq],
                                    mt[:, 2 * p : 2 * p + 2, nb * MM_N : (nb + 1) * MM_N],
                                    start=(p == 0),
                                    stop=(p == 1),
                                    perf_mode=DR,
                                    skip_group_check=True,
                                )
                        step = plan[s]
                        if step == "A":
                            arr = epool.tile(
                                [P, STRIP], f16, tag=f"e{len(arrays[t])}"
                            )
                            nc.scalar.activation(arr, ps, COPY)
                            arrays[t].append(arr)
                        else:
                            out = fpool.tile([P, STRIP], f16, tag="f")
                            nc.vector.tensor_max(out, ps, arrays[t][step])
                            arrays[t][step] = out

                for t in pair:
                    arrs = arrays[t]
                    # balanced DVE 2x fp16 merge tree down to one array
                    while len(arrs) > 1:
                        nxt = []
                        for i in range(0, len(arrs) - 1, 2):
                            o = gpool.tile([P, STRIP], f16, tag="g")
                            nc.vector.tensor_max(o, arrs[i], arrs[i + 1])
                            nxt.append(o)
                        if len(arrs) % 2:
                            nxt.append(arrs[-1])
                        arrs = nxt

                    # norm-bias add, split Pool (first 512) / DVE (rest) so
                    # both finish together, then cross-group bucket fold
                    # 2048 -> 1024 and hardware max8
                    HALF = STRIP // 2
                    PW = 512  # Pool's slice of the bias add
                    folded = arrs[0]
                    scored = epool.tile([P, STRIP], f16, tag="scored")
                    nc.gpsimd.tensor_tensor(
                        scored[:, :PW], folded[:, :PW], m2t[:, :PW], op=ADD
                    )
                    nc.vector.tensor_tensor(
                        scored[:, PW:], folded[:, PW:], m2t[:, PW:], op=ADD
                    )
                    sc1 = epool.tile([P, HALF], f16, tag="sc1")
                    nc.vector.tensor_max(
                        sc1, scored[:, :HALF], scored[:, HALF:]
                    )
                    o8 = opool.tile([P, 8], f16, tag="o8")
                    nc.vector.max(out=o8, in_=sc1)
                    nc.sync.dma_start(out=c8[t], in_=o8)

    return nc


def _host_inputs(phi_p, memory_bank):
    """Build per-core input maps (fp8 queries/bank, sorted-norm layout)."""
    x = np.ascontiguousarray(phi_p.reshape(B, C, H * W))  # [4, 512, 4096]

    m2 = (memory_bank.astype(np.float64) ** 2).sum(axis=1)  # [N_BANK]
    order = np.argsort(m2, kind="stable")
    m_sorted = memory_bank[order]                  # rank r -> bank row
    t_sorted = (-0.5 * m2[order] + SHIFT).astype(np.float64)

    # rank r lives at bank column n = (r % NSTRIP)*STRIP + r//NSTRIP
    ranks = np.arange(N_BANK)
    cols = (ranks % NSTRIP) * STRIP + ranks // NSTRIP
    m_laid = np.empty_like(m_sorted)
    m_laid[cols] = m_sorted                        # [N_BANK, C] in device order

    mq = np.ascontiguousarray(
        m_laid.T.reshape(KC, P, N_BANK).transpose(1, 0, 2)
    ).astype(ml_dtypes.float8_e4m3)

    group_bias = t_sorted.reshape(STRIP, GROUP).mean(axis=1).astype(np.float16)
    m2g = np.broadcast_to(group_bias, (P, STRIP)).copy()

    in_maps = []
    for i in range(N_CORES):
        b = i // 2
        lo = (i % 2) * Q_PER_CORE
        xT_i = x[b][:, lo : lo + Q_PER_CORE]       # [512, 2048]
        xq_i = np.ascontiguousarray(
            xT_i.reshape(KC, P, Q_PER_CORE).transpose(1, 0, 2)
        ).astype(ml_dtypes.float8_e4m3)
        in_maps.append({"xq": xq_i, "mq": mq, "m2g": m2g})
    return in_maps


def _finish_loss(phi_p, r, c8_all):
    """c8_all: [16384, 8] descending top-8 of dot - 0.5||m||^2 + SHIFT."""
    x2 = (phi_p.astype(np.float64) ** 2).sum(axis=1).reshape(Q_TOTAL)  # (b, hw)
    d2 = x2[:, None] - 2.0 * (c8_all[:, : K + J].astype(np.float64) - SHIFT)
    d2 = np.maximum(d2, 0.0)                       # ascending
    r2 = float(r[0]) ** 2
    loss_att = np.mean(np.maximum(d2[:, :K] - r2, 0.0)) / NU
    loss_rep = np.mean(np.maximum(r2 - d2[:, J:] - ALPHA, 0.0)) / NU
    return np.array(loss_att + loss_rep, dtype=np.float32)


def run_device(in_maps, trace=False):
    from concourse.bass_utils import run_bass_kernel_spmd

    nc = build_program()
    if not nc.is_finalized():
        nc.finalize()
    return run_bass_kernel_spmd(nc, in_maps, list(range(N_CORES)), trace=trace)


def kernel(phi_p, memory_bank, r):
    phi_p = np.asarray(phi_p, dtype=np.float32)
    memory_bank = np.asarray(memory_bank, dtype=np.float32)
    r = np.asarray(r, dtype=np.float32)
    in_maps = _host_inputs(phi_p, memory_bank)
    res = run_device(in_maps)
    c8_all = np.concatenate(
        [
            np.asarray(res.results[i]["c8"]).astype(np.float32).reshape(Q_PER_CORE, 8)
            for i in range(N_CORES)
        ],
        axis=0,
    )
    return _finish_loss(phi_p, r, c8_all)
